# revision 51
# baseline (speedup 1.0000x reference)
"""Trainium2 Bass kernel for nn_DevConvLayer (gnn_message_passing).

Reference math:
    s = x.sum(1)                       # [N]
    T = (s[:,None] - s[None,:]) * A    # [N,N]
    M = max(T*wmax, T*wmin).max(1)     # [N]   wmax/wmin = col stats of W_phi
    out = broadcast(where(deg>0, M, 0), [N,3])

Kernel restructure (exact up to ~1e-2 abs, tolerance is 5.6e-2):
  * wmax >= 0 and the j==i candidate is always 0, so
    M[i] = relu(max_j A_ij * wmax_j * (s_i - s_j)).  Only columns with
    s_j < s_i can produce a positive candidate.
  * HOST-side: sort nodes by s ascending; permute columns of A by that
    order and deal rows round-robin (rank mod 8) to the 8 cores.  Then the
    k-th sorted row of a core only needs the first ~8k+c columns -> ~44%
    of all DMA / PE / reduce work is provably skippable.
  * A ships as fp8e4 {0,1} (4.6 MiB/core after pruning) instead of int32
    (32 MiB/core).
  * ONE DoubleRow fp8 matmul per 512-col tile computes
        psum[i,j] = C*A_ij + (s_i*w_j - v_j - C),   C=16, v=w*s
    64 lhsT partitions carry a paired diagonal (the mask term C*A), 5 more
    carry rank-10 fp8-split terms.  DoubleRow runs 0.5 cycles/row.
  * Drain (the bottleneck) uses all three ALU engines:
      - right region (row-major [i,j] tiles): Act copies psum->bf16 stage,
        DVE fold-maxes the stage (2x mode) or DVE TensorReduces psum (1x);
      - left region (columns j < JT, computed TRANSPOSED as [j,i] tiles):
        Act copies psum->bf16, the Pool engine does the C-axis (partition)
        max-reduce into per-tile partial rows, exported raw via `outl`;
        the host max-combines them with the row-region result (a 16x1024
        shard-stitch, like the per-core output concat).
    Non-neighbours sit at Y-C <= -13 < 0 and lose to the final relu.

Sharding: rows dealt round-robin by s-rank; columns globally sorted;
W_phi stats replicated.  Host prep is O(N^2) byte shuffling only.
"""

import numpy as np
import ml_dtypes

import concourse.bass as bass
import concourse.mybir as mybir
import concourse.tile as tile
from concourse.bass_utils import run_bass_kernel_spmd
from concourse.tile import add_dep_helper

N_CORES = 8
N = 8192
IN_CH = 3
P = 128
RB = 8                  # row blocks per core
ROWS = RB * P           # rows per core
KP = 69                 # matmul contraction partitions: 64 diag pairs + 5 Y
C_BIAS = 16.0

F32 = mybir.dt.float32
BF16 = mybir.dt.bfloat16
FP8 = mybir.dt.float8e4

AX = mybir.AxisListType
OP = mybir.AluOpType
AF = mybir.ActivationFunctionType

F8NP = ml_dtypes.float8_e4m3

CFG = {
    "jbT": 16,          # transposed column blocks (left region = 128*jbT cols)
    "fold_delay": 1,
    "force_dve_r": 9,   # first N right units forced to the DVE path so the
                        # Act queue front-loads the left-region copies
    "debug": False,
    "dve_l_jobs": (),
    "force_dve_tail": 0,
}

JT = lambda: 128 * CFG["jbT"]

# measured per-op costs (ns) for the static drain balancer
C_ACT_1K = 1040.0       # act copy [128,1024] psum->bf16
C_DVE_TR_1K = 1280.0    # DVE tensor_reduce [128,1024] psum
C_DVE_FOLD_2K = 1480.0  # DVE fold chain on [128,2048] stage
C_DVE_FOLD_1K = 920.0   # DVE fold chain on [128,1024] stage
C_POOL_CTR = lambda w: w * 1.53 + 140.0    # pool C-axis reduce width w (fudged)
C_ACT = lambda w: w * 0.8333 + 217.0       # act copy width w
C_DVE_MERGE = lambda w: w * 1.0417 + 190.0 # DVE TT psum->bf16 acc width w


def _right_units():
    """Row-major drain units (rb, col0, n_tiles) covering j in [JT, W_rb)."""
    units = []
    for rb in range(RB):
        W = 1024 * (rb + 1)
        col = JT()
        while col < W:
            nt = 2 if W - col >= 2048 else 1
            units.append((rb, col, nt))
            col += nt * 1024
    return units


def _plan():
    """Static schedule: left jb-tiles (Act+Pool or DVE-merge) and right
    units (Act+DVE-fold or DVE-TR), greedily balanced."""
    busy = {"ACT": 0.0, "DVE": 0.0, "POOL": 0.0}
    jobs = []
    # 2 left jobs per right job: left finishes ~2/3 in, so its DRAM
    # round-trip collection chain overlaps the remaining right work
    left = [("L", jb) for jb in range(CFG["jbT"])]
    right = [("R", u) for u in _right_units()]
    seq = []
    li, ri = 0, 0
    while li < len(left) or ri < len(right):
        for _ in range(CFG.get("l_per_r", 2)):
            if li < len(left):
                seq.append(left[li]); li += 1
        if ri < len(right):
            seq.append(right[ri]); ri += 1
    for kind, info in seq:
        if kind == "L":
            jb = info
            w = 1024 - 16 * jb
            if jb in CFG["dve_l_jobs"]:
                jobs.append(("L", jb, "DVE"))
                busy["DVE"] += C_DVE_MERGE(w)
            else:
                jobs.append(("L", jb, "ACT"))
                busy["ACT"] += C_ACT(w)
                busy["POOL"] += C_POOL_CTR(w)
        else:
            rb, col, nt = info
            act_a = busy["ACT"] + nt * C_ACT_1K
            dve_a = busy["DVE"] + (C_DVE_FOLD_2K if nt == 2 else C_DVE_FOLD_1K)
            mx_a = max(act_a, dve_a)
            dve_d = busy["DVE"] + nt * C_DVE_TR_1K
            mx_d = max(busy["ACT"], dve_d)
            n_r_so_far = sum(1 for j in jobs if j[0] == "R")
            n_r_total = len(right)
            tail_force = n_r_so_far >= n_r_total - CFG.get("force_dve_tail", 0)
            if n_r_so_far < CFG["force_dve_r"] or tail_force or mx_a > mx_d:
                jobs.append(("R", info, "DVE"))
                busy["DVE"] = dve_d
            else:
                jobs.append(("R", info, "ACT"))
                busy["ACT"], busy["DVE"] = act_a, dve_a
    return jobs, busy


def _emit(ctx, tc, aps):
    nc = tc.nc
    a_ap = aps["a8"]
    at_ap = aps["a8t"]
    lt_ap = aps["lt"]
    ltT_ap = aps["ltT"]
    out_ap = aps["outp"]
    JB = CFG["jbT"]

    # ---- semaphore hygiene (same preamble as the known-good baseline) ----
    from concourse.bass import compact_to_ranges
    clear_prev = None
    for sem_range in compact_to_ranges(
        [s for s in nc._kernel_sem_range if s not in nc.barrier_sems]
    ):
        i1 = nc.gpsimd.dma_reset(sem_range)
        if clear_prev is not None:
            add_dep_helper(i1.ins, clear_prev.ins, False, "clear order")
        i2 = nc.gpsimd.sem_clear(sem_range)
        add_dep_helper(i2.ins, i1.ins, False, "clear order")
        clear_prev = i2
    for engine in nc.engines.values():
        pb = engine.isa(
            nc.isa.Opcode.NEURON_ISA_TPB_OPCODE_PSEUDO_SYNC_BARRIER,
            {},
            struct_name="NEURON_ISA_TPB_UNKNOWN_STRUCT",
            verify=False,
        )
        if clear_prev is not None:
            add_dep_helper(pb.ins, clear_prev.ins, False, "barrier after clear")
    tc.no_sync_barrier()

    prep = ctx.enter_context(tc.tile_pool(name="prep", bufs=1))
    psumL = psumR = ctx.enter_context(
        tc.tile_pool(name="psum", bufs=4, space="PSUM")
    )
    stgp = ctx.enter_context(tc.tile_pool(name="stg", bufs=4))
    stgT = ctx.enter_context(tc.tile_pool(name="stgT", bufs=8))
    fold = ctx.enter_context(tc.tile_pool(name="fold", bufs=2))
    dram = ctx.enter_context(tc.tile_pool(name="dram", bufs=1, space="DRAM"))

    # ---- aux loads.  Left-region aux rides the Pool SWDGE queue (no
    # HWDGE contention); SP keeps the A streams.  The whole transposed A^T
    # lives resident in one [69, 2, JB, 1024] tile, loaded in 3 chunks so
    # jb0 can start early; sr rows are host-replicated per jb.
    lt = prep.tile([KP, 2, RB, P], FP8)
    ltT = prep.tile([KP, 2, max(JB, 1), P], FP8)
    a_t = [prep.tile([KP, 2, N], FP8, tag=f"ab{i}", name=f"ab{i}")
           for i in range(2)]
    # a8t / a8 arrive PACKED: 138 = 69*2 pair-rows per block, rows 128..137
    # carrying the sr / wv rhs rows, so ONE DMA covers partitions 0..68 and
    # HWDGE slots (632ns each, globally serialized) are conserved.
    AT_GROUPS = [(0, 1), (1, 2), (2, 4), (4, 8), (8, 12), (12, 16), (16, 20), (20, 24), (24, 28), (28, 32)]
    AT_GROUPS = [(a, min(b, JB)) for a, b in AT_GROUPS if a < JB]
    aTg = {
        g: prep.tile([KP, g[1] - g[0], 2, 1024], FP8, tag=f"aT{g[0]}",
                     name=f"aT{g[0]}")
        for g in AT_GROUPS
    }
    grp_of = {}
    for g in AT_GROUPS:
        for jb in range(g[0], g[1]):
            grp_of[jb] = g

    def a8t_chunk(g, eng=None):
        j0, j1 = g
        (eng or nc.sync).dma_start(
            aTg[g][:, :, :, :],
            at_ap[j0 * 138 : j1 * 138, :].rearrange(
                "(jb k t) i -> k jb (t i)", k=KP, t=2
            ),
        )

    def r_chunk(rb):
        W = 1024 * (rb + 1)
        nc.sync.dma_start(
            a_t[rb % 2][:, :, JT() : W],
            a_ap[rb * 138 : (rb + 1) * 138, JT() : W].rearrange(
                "(k t) j -> k t j", t=2
            ),
        )

    if JB:
        nc.scalar.dma_start(
            ltT[:].rearrange("k t b m -> k (t b m)"),
            ltT_ap.rearrange("k t b m -> k (t b m)"),
        )
        a8t_chunk(AT_GROUPS[0], eng=nc.scalar)
    for g in AT_GROUPS[1:3]:
        a8t_chunk(g)
    first_rb = JT() // 1024
    rdma_done_pre = set()
    if first_rb < RB:
        r_chunk(first_rb)
        rdma_done_pre.add(first_rb)
    nc.sync.dma_start(
        lt[:].rearrange("k t r m -> k (t r m)"),
        lt_ap.rearrange("k t r m -> k (t r m)"),
    )
    for g in AT_GROUPS[3:4]:
        a8t_chunk(g)
    if first_rb + 1 < RB:
        r_chunk(first_rb + 1)
        rdma_done_pre.add(first_rb + 1)
    for g in AT_GROUPS[4:]:
        a8t_chunk(g)

    partials = prep.tile([P, RB, 8], F32)
    nc.vector.memset(partials[:], -100000.0)
    dev = prep.tile([P, RB], F32)
    # left partial rows: one [1,1024] row per jb tile, exported raw via
    # outl; the host max-combines them (gap columns i<16*jb are masked
    # host-side, so prow needs no initialisation)
    prow = prep.tile([1, max(JB, 1) * 1024], F32)
    arow = prep.tile([1, 1024], F32)
    accT = prep.tile([P, 1024], BF16)
    if JB and CFG["dve_l_jobs"]:
        nc.vector.memset(accT[:], -100000.0)

    jobs, busy = _plan()

    slot_of = [0] * RB
    pending_fold = []
    rdma_done = set(rdma_done_pre)
    n_left_done = 0
    dve_left_used = any(k == "L" and p == "DVE" for k, i, p in jobs)
    n_left_dve_total = 0
    seen = 0
    for k, i, p in jobs:
        if k == "L":
            seen += 1
            if p == "DVE":
                n_left_dve_total = seen

    def flush_fold():
        if not pending_fold:
            return
        stg, wS, rb_, slot_ = pending_fold.pop(0)
        w = wS
        cur = stg
        while w > 256:
            h = w // 2
            nx = fold.tile([P, 1024], BF16, tag=f"f{h}", name=f"f{h}")
            nc.vector.tensor_tensor(nx[:, 0:h], cur[:, 0:h], cur[:, h:w], OP.max)
            cur, w = nx, h
        nc.vector.tensor_reduce(
            partials[:, rb_, slot_ : slot_ + 1], cur[:, 0:w], AX.X, OP.max
        )

    for kind, info, path in jobs:
        if kind == "L":
            jb = info
            i0 = 16 * jb
            w = 1024 - i0
            g = grp_of[jb]
            pg = psumL.tile([P, 1024], F32, tag="pg", name="pg")
            # slice at the 512 psum-bank boundary: a straddling matmul's
            # start=True zeroing clobbers the neighbour slice's cells
            for c0, c1 in ((i0, 512), (512, 1024)):
                nc.tensor.matmul(
                    pg[:, c0:c1],
                    ltT[:, :, jb, :],
                    aTg[g][:, jb - g[0], :, c0:c1],
                    start=True,
                    stop=True,
                    perf_mode=mybir.MatmulPerfMode.DoubleRow,
                    skip_group_check=True,
                )
            if path == "ACT":
                st = stgT.tile([P, 1024], BF16, tag="stT", name="stT")
                nc.scalar.activation(st[:, i0:1024], pg[:, i0:1024], AF.Copy)
                nc.gpsimd.tensor_reduce(
                    prow[:, jb * 1024 + i0 : (jb + 1) * 1024],
                    st[:, i0:1024],
                    AX.C,
                    OP.max,
                )
                if CFG["debug"] and jb == 0:
                    dst = prep.tile([P, 1024], F32, tag="dbgstg", name="dbgstg")
                    nc.vector.tensor_copy(dst[:], st[:])
                    nc.sync.dma_start(aps["dbg_stg"], dst[:])
            if path != "ACT":
                nc.vector.tensor_tensor(
                    accT[:, i0:1024], pg[:, i0:1024], accT[:, i0:1024], OP.max
                )
            n_left_done += 1
            if dve_left_used and n_left_done == n_left_dve_total:
                # accT complete: C-reduce it and fold into prow's jb0 slot
                nc.gpsimd.tensor_reduce(arow[:], accT[:], AX.C, OP.max)
                nc.vector.tensor_tensor(
                    prow[:, 0:1024], prow[:, 0:1024], arow[:], OP.max
                )
            if n_left_done == JB:
                nc.sync.dma_start(aps["outl"], prow[:])
        else:
            rb, col, nt = info
            at = a_t[rb % 2]
            lhsT = lt[:, :, rb, :]
            wS = nt * 1024
            if rb not in rdma_done:
                rdma_done.add(rb)
                r_chunk(rb)
            ptiles = []
            for t in range(nt):
                pg = psumR.tile([P, 1024], F32, tag="pg", name="pg")
                for h in range(2):
                    j0 = col + t * 1024 + h * 512
                    nc.tensor.matmul(
                        pg[:, h * 512 : (h + 1) * 512],
                        lhsT,
                        at[:, :, j0 : j0 + 512],
                        start=True,
                        stop=True,
                        perf_mode=mybir.MatmulPerfMode.DoubleRow,
                        skip_group_check=True,
                    )
                ptiles.append(pg)
            slot = slot_of[rb]
            if path == "ACT":
                stg = stgp.tile([P, 2048], BF16, tag="stg", name="stg")
                for t, pg in enumerate(ptiles):
                    nc.scalar.activation(
                        stg[:, t * 1024 : (t + 1) * 1024], pg[:], AF.Copy
                    )
                pending_fold.append((stg, wS, rb, slot))
                slot_of[rb] += 1
                while len(pending_fold) > CFG["fold_delay"]:
                    flush_fold()
            else:
                for pg in ptiles:
                    nc.vector.tensor_reduce(
                        partials[:, rb, slot : slot + 1], pg[:], AX.X, OP.max
                    )
                    slot += 1
                slot_of[rb] = slot
                flush_fold()
    while pending_fold:
        flush_fold()

    # ---- finalize ----
    nc.vector.tensor_reduce(dev[:], partials[:], AX.X, OP.max)
    nc.vector.tensor_scalar_max(dev[:], dev[:], 0.0)
    # out[p*RB + g] = dev[p, g]; host untangles the order
    nc.sync.dma_start(out_ap.rearrange("(p g) -> p g", g=RB), dev[:])


def _legalize_waits(nc, max_sems=1):
    """Walrus codegen accepts at most one semaphore wait per instruction.
    Hoist every excess wait onto an InstEventSemaphore inserted just before
    the instruction on the same engine stream."""
    n_new = 0
    for fn in nc.m.functions:
        for blk in fn.blocks:
            insts = blk.instructions
            out = []
            for inst in insts:
                si = inst.sync_info
                if si is not None and si.on_wait:
                    by_sem = {}
                    order = []
                    for w in si.on_wait:
                        if w.id not in by_sem:
                            by_sem[w.id] = w
                            order.append(w.id)
                        elif (w.wait_value or 0) > (by_sem[w.id].wait_value or 0):
                            by_sem[w.id] = w
                    if len(order) > max_sems or len(by_sem) != len(si.on_wait):
                        keep = order[-max_sems:]
                        for sid in order[: len(order) - max_sems]:
                            ev = mybir.InstEventSemaphore(
                                name=f"hoist_{nc.next_id()}", ins=[], outs=[]
                            )
                            ev.engine = inst.engine
                            ev.sync_info = mybir.SyncInfo(
                                on_wait=[by_sem[sid]], on_update=[]
                            )
                            out.append(ev)
                            n_new += 1
                        inst.sync_info = mybir.SyncInfo(
                            on_wait=[by_sem[s] for s in keep],
                            on_update=list(si.on_update),
                        )
                out.append(inst)
            insts[:] = out
    return n_new


def build_nc(rows=ROWS, cols=N, legalize=True):
    from contextlib import ExitStack

    JB = CFG["jbT"]
    nc = bass.Bass(
        "TRN2", target_bir_lowering=False, debug=False, num_devices=N_CORES
    )
    aps = {
        "a8": nc.dram_tensor(
            "a8", [RB * 138, cols], FP8, kind="ExternalInput"
        ).ap(),
        "a8t": nc.dram_tensor(
            "a8t", [max(JB, 1) * 138, 1024], FP8, kind="ExternalInput"
        ).ap(),
        "lt": nc.dram_tensor("lt", [KP, 2, RB, P], FP8, kind="ExternalInput").ap(),
        "ltT": nc.dram_tensor(
            "ltT", [KP, 2, max(JB, 1), P], FP8, kind="ExternalInput"
        ).ap(),
        "outp": nc.dram_tensor("outp", [ROWS], F32, kind="ExternalOutput").ap(),
        "outl": nc.dram_tensor(
            "outl", [max(JB, 1) * 1024], F32, kind="ExternalOutput"
        ).ap(),
    }
    if CFG["debug"]:
        aps["dbg_prow"] = nc.dram_tensor(
            "dbg_prow", [max(JB, 1) * 1024], F32, kind="ExternalOutput"
        ).ap()
        aps["dbg_ldev"] = nc.dram_tensor(
            "dbg_ldev", [ROWS], F32, kind="ExternalOutput"
        ).ap()
        aps["dbg_stg"] = nc.dram_tensor(
            "dbg_stg", [P, 1024], F32, kind="ExternalOutput"
        ).ap()
        aps["dbg_pcol"] = nc.dram_tensor(
            "dbg_pcol", [max(JB, 1), 1024], F32, kind="ExternalOutput"
        ).ap()
        aps["dbg_lrow"] = nc.dram_tensor(
            "dbg_lrow", [1024], F32, kind="ExternalOutput"
        ).ap()
    with tile.TileContext(nc) as tc:
        with ExitStack() as ctx:
            _emit(ctx, tc, aps)
    if legalize:
        _legalize_waits(nc)
    return nc


def _split3(x):
    p0 = x.astype(F8NP)
    r = x - p0.astype(np.float32)
    p1 = r.astype(F8NP)
    p2 = (r - p1.astype(np.float32)).astype(F8NP)
    return (
        p0.astype(np.float32),
        p1.astype(np.float32),
        p2.astype(np.float32),
    )


def make_in_maps(x, adjacency_matrix, W_phi, n_cores=N_CORES):
    x = np.asarray(x, dtype=np.float32)
    A = np.asarray(adjacency_matrix)
    W = np.asarray(W_phi, dtype=np.float32)
    JB = CFG["jbT"]

    s = x.sum(axis=1)
    order = np.argsort(s, kind="stable")
    w = W.max(axis=0)
    v = w * s

    A8_cols = A.astype(np.int8)[:, order].astype(F8NP)
    s_sorted = s[order]
    w_sorted = w[order]
    v_sorted = v[order]

    w0, w1, w2 = _split3(w_sorted)
    v0, v1, v2 = _split3(v_sorted)
    ones_c = np.ones(N, np.float32)
    wv = np.zeros((5, 2, N), np.float32)
    wv[0, 0], wv[0, 1] = w0, w1            # pair with (s0, s0)
    wv[1, 0], wv[1, 1] = w0, w1            # pair with (s1, s1)
    wv[2, 0], wv[2, 1] = w2, w0            # pair with (s0, s2)
    wv[3, 0], wv[3, 1] = -v0, -v1          # pair with (1, 1)
    wv[4, 0], wv[4, 1] = -v2, -C_BIAS * ones_c   # pair with (1, 1)
    wv8 = wv.astype(F8NP)

    # transposed-region lhsT columns: w/v pieces per jb block
    ltT = np.zeros((KP, 2, max(JB, 1), P), np.float32)
    for k in range(64):
        for t in range(2):
            ltT[k, t, :, 2 * k + t] = C_BIAS
    if JB:
        wb = lambda p: p[: JB * P].reshape(JB, P)
        ltT[64, 0, :, :], ltT[64, 1, :, :] = wb(w0), wb(w0)
        ltT[65, 0, :, :], ltT[65, 1, :, :] = wb(w1), wb(w1)
        ltT[66, 0, :, :], ltT[66, 1, :, :] = wb(w2), wb(w0)
        ltT[67, 0, :, :], ltT[67, 1, :, :] = -wb(v0), -wb(v1)
        ltT[68, 0, :, :] = -wb(v2)
        ltT[68, 1, :, :] = -C_BIAS
    ltT8 = ltT.astype(F8NP)

    in_maps = []
    for c in range(n_cores):
        rows_c = order[c::n_cores]
        a8 = np.ascontiguousarray(A8_cols[rows_c])
        a8t = np.ascontiguousarray(a8[:, : max(JB, 1) * P].T)
        s_c = s_sorted[c::n_cores]
        s0, s1, s2 = _split3(s_c)
        ones_p = np.ones(ROWS, np.float32)
        lhsT = np.zeros((KP, 2, RB, P), np.float32)
        for k in range(64):
            for t in range(2):
                lhsT[k, t, :, 2 * k + t] = C_BIAS
        def put(kk, tt, vec):
            lhsT[kk, tt] = vec.reshape(RB, P)
        put(64, 0, s0); put(64, 1, s0)
        put(65, 0, s1); put(65, 1, s1)
        put(66, 0, s0); put(66, 1, s2)
        put(67, 0, ones_p); put(67, 1, ones_p)
        put(68, 0, ones_p); put(68, 1, ones_p)
        # transposed-region rhs s-rows, replicated per jb
        sr1 = np.zeros((5, 2, 1024), np.float32)
        sr1[0, 0], sr1[0, 1] = s0, s1
        sr1[1, 0], sr1[1, 1] = s0, s1
        sr1[2, 0], sr1[2, 1] = s0, s2
        sr1[3, 0], sr1[3, 1] = ones_p, ones_p
        sr1[4, 0], sr1[4, 1] = ones_p, ones_p

        # pack: per block, 128 pair-rows of A then the 10 rhs aux rows
        wv10 = wv8.reshape(10, N)
        a8p = np.empty((RB * 138, N), F8NP)
        for rb in range(RB):
            a8p[rb * 138 : rb * 138 + 128] = a8[rb * 128 : (rb + 1) * 128]
            a8p[rb * 138 + 128 : (rb + 1) * 138] = wv10
        sr10 = sr1.astype(F8NP).reshape(10, 1024)
        nJB = max(JB, 1)
        a8tp = np.empty((nJB * 138, 1024), F8NP)
        for jb in range(nJB):
            a8tp[jb * 138 : jb * 138 + 128] = a8t[jb * 128 : (jb + 1) * 128]
            a8tp[jb * 138 + 128 : (jb + 1) * 138] = sr10
        in_maps.append(
            {
                "a8": a8p,
                "a8t": a8tp,
                "lt": lhsT.astype(F8NP),
                "ltT": ltT8,
            }
        )
    return in_maps, order


_NC_CACHE = {}


def _get_nc():
    if "nc" not in _NC_CACHE:
        _NC_CACHE["nc"] = build_nc()
    return _NC_CACHE["nc"]


def kernel(**inputs) -> np.ndarray:
    x = np.asarray(inputs["x"])
    A = inputs["adjacency_matrix"]
    W_phi = inputs["W_phi"]
    nc = _get_nc()
    in_maps, order = make_in_maps(x, A, W_phi)
    # warm-up execution for deterministic semaphore state (see baseline)
    run_bass_kernel_spmd(nc, in_maps, list(range(N_CORES)))
    res = run_bass_kernel_spmd(nc, in_maps, list(range(N_CORES)))
    full = np.empty(N, np.float32)
    JB = CFG["jbT"]
    for c in range(N_CORES):
        arr = res.results[c]["outp"].reshape(P, RB)   # outp[p*RB+g]=dev[p,g]
        loc = arr.T.reshape(-1).copy()                # local k = g*128+p
        if JB:
            outl = res.results[c]["outl"].reshape(JB, 1024)
            lm = np.full(1024, -np.inf, np.float32)
            for jb in range(JB):
                i0 = 16 * jb
                lm[i0:] = np.maximum(lm[i0:], outl[jb, i0:])
            loc = np.maximum(loc, np.maximum(lm, 0.0).astype(np.float32))
        full[order[c::N_CORES]] = loc
    out = np.broadcast_to(full[:, None], (N, IN_CH)).astype(np.float32)
    return np.ascontiguousarray(out)
